# revision 13
# baseline (speedup 1.0000x reference)
"""GTrajRecovery Trainium2 Bass kernel.

Full inputs in, full outputs out.  Internally shards the batch (B=256)
across 8 NeuronCores (32 samples each); weights are replicated.

Structure per core:
  phase 1a: GAT over per-sample chain graphs + LayerNorm -> route_outputs
  phase 1b: gather emb_id rows, transpose, batched gi = x @ W_ih^T
  phase 2 : 255 sequential GRU-cell steps (only the cell is serial)
  phase 3a: attention scores/softmax/weighted, batched over all steps
  phase 3b: fc1/relu/fc2/sigmoid rate head, batched over all tokens
"""

import os
import sys
import numpy as np
from contextlib import ExitStack

os.environ.setdefault("MYCRO_LOCAL_CACHE", "1")
if "/opt/trn_rl_repo" not in sys.path:
    sys.path.insert(0, "/opt/trn_rl_repo")

# ---- problem constants (hardcoded per contract) ----
H = 256
HEADS = 4
DH = 64
B = 256
L = 128
T = 256
ID = 50000
NCORES = 8
BL = B // NCORES          # 32 samples per core
NSTEP = T - 1             # 255 sequential steps
TOK = NSTEP * BL          # 8160 tokens per core
G = 6                     # 128-row gate blocks in 3H
CS_GI = 510               # token chunk for gi / fc (8160 = 16*510)
NCH_GI = TOK // CS_GI     # 16
GIB = 16                  # gi batch (steps) streamed in phase 2
NGT = (TOK + 127) // 128  # 64 gather tiles (last padded)

F32 = None  # filled lazily with mybir dtypes
_BUILD_CACHE = {}


def _emit(ctx, tc, io, flags):
    import concourse.bass as bass
    from concourse import mybir
    from concourse.masks import make_identity

    nc = tc.nc
    dt = mybir.dt
    Alu = mybir.AluOpType
    Act = mybir.ActivationFunctionType
    AX = mybir.AxisListType

    f32 = dt.float32
    f32r = dt.float32r
    bf16 = dt.bfloat16

    rec_bf = flags["recur_bf16"]
    rec_mm_dt = bf16 if rec_bf else f32

    def r32(ap):
        return ap.bitcast(f32r)

    # DRAM APs
    d_remb = io["route_emb"].ap()        # [BL, L, H] f32
    d_idx = io["trg_idx"].ap()           # [128, NGT] i32
    d_rate = io["trg_rate"].ap()         # [1, TOK] f32
    d_emb = io["emb_id"].ap()            # [ID, H] f32
    d_gatw = io["gat_w"].ap()            # [H, H]
    d_aflat = io["gat_a"].ap()           # [1, 2H] (a_src_exp | a_dst_exp)
    d_gbias = io["gat_bias"].ap()        # [1, H]
    d_gamma = io["ln_gamma"].ap()        # [1, H]
    d_beta = io["ln_beta"].ap()          # [1, H]
    d_whh = io["w_hhT"].ap()             # [H, 3H]
    d_wih = io["w_ihT"].ap()             # [H, 3H]
    d_ratecol = io["rate_col"].ap()      # [128, G]
    d_fc1 = io["fc1_wT"].ap()            # [2H, 2H]
    d_fc1b = io["fc1_b"].ap()            # [128, 4]
    d_fc2 = io["fc2_wT"].ap()            # [128, 4]
    d_fc2b = io["fc2_b"].ap()            # [1, 1]
    d_oid = io["out_id"].ap()            # [T, BL, L]
    d_orate = io["out_rate"].ap()        # [T, BL, 1]
    d_gi = io["gi_scratch"].ap()         # [G, 128, TOK]
    d_et = io["et_scratch"].ap()         # [2, 128, NGT*128]

    has_gbias = flags["has_gat_bias"]
    has_beta = flags["has_ln_beta"]
    has_bias_rz = flags["has_bias_rz"]   # (b_ih + b_hh)[:512] nonzero
    has_bias_in = flags["has_bias_in"]   # b_ih[512:] nonzero
    has_bias_hn = flags["has_bias_hn"]   # b_hh[512:] nonzero
    d_bias_rz = io["bias_rz"].ap()       # [128, 4]
    d_bias_in = io["bias_in"].ap()       # [128, 2]
    d_bias_hn = io["bias_hn"].ap()       # [128, 2]

    gi_view = d_gi.rearrange("g p n -> p g n")   # [128, G, TOK]

    # ---------------- constant pool (whole kernel) ----------------
    consts = ctx.enter_context(tc.tile_pool(name="consts", bufs=1))

    ident = consts.tile([128, 128], f32)
    make_identity(nc, ident[:])

    whh_sb = consts.tile([128, 2, 3 * H], f32)
    nc.sync.dma_start(out=whh_sb[:], in_=d_whh.rearrange("(k p) j -> p k j", p=128))
    whh_bf = None
    if rec_bf:
        whh_bf = consts.tile([128, 2, 3 * H], bf16)
        nc.vector.tensor_copy(out=whh_bf[:], in_=whh_sb[:])

    ratecol_sb = consts.tile([128, G], f32)
    nc.sync.dma_start(out=ratecol_sb[:], in_=d_ratecol)
    fc2T_sb = consts.tile([128, 4], f32)
    nc.sync.dma_start(out=fc2T_sb[:], in_=d_fc2)
    fc1b_sb = consts.tile([128, 4], f32)
    nc.sync.dma_start(out=fc1b_sb[:], in_=d_fc1b)
    fc2b_sb = consts.tile([1, 1], f32)
    nc.sync.dma_start(out=fc2b_sb[:], in_=d_fc2b)
    idx_sb = consts.tile([128, NGT], dt.int32)
    nc.sync.dma_start(out=idx_sb[:], in_=d_idx)
    eps_sb = consts.tile([128, 1], f32)
    nc.vector.memset(eps_sb[:], 1e-5)
    brz_sb = bin_sb = bhn_sb = None
    if has_bias_rz:
        brz_sb = consts.tile([128, 4], f32)
        nc.sync.dma_start(out=brz_sb[:], in_=d_bias_rz)
    if has_bias_in:
        bin_sb = consts.tile([128, 2], f32)
        nc.sync.dma_start(out=bin_sb[:], in_=d_bias_in)
    if has_bias_hn:
        bhn_sb = consts.tile([128, 2], f32)
        nc.sync.dma_start(out=bhn_sb[:], in_=d_bias_hn)

    # ---------------- long-lived big tensors ----------------
    big = ctx.enter_context(tc.tile_pool(name="big", bufs=1))
    # slot "sA": xw_C_all -> ET_all -> (none)   max 8.4MB
    # slot "sB": nodes_all -> H_allT            max 8.3MB
    # slot "sC": (1a tmp) -> wT_all
    ro_C = big.tile([128, BL, H], f32, tag="roC")        # route_outputs, [l, b, h]
    roT = big.tile([128, 2, BL, L], f32, tag="roT")      # (1/16)-scaled transpose

    evac_flip = [0]

    def evac_copy(out, in_, scale=None):
        """psum -> sbuf eviction, alternating DVE/ACT."""
        evac_flip[0] ^= 1
        if scale is None:
            if evac_flip[0]:
                nc.scalar.copy(out=out, in_=in_)
            else:
                nc.vector.tensor_copy(out=out, in_=in_)
        else:
            if evac_flip[0]:
                nc.scalar.mul(out=out, in_=in_, mul=scale)
            else:
                nc.vector.tensor_scalar_mul(out=out, in0=in_, scalar1=scale)

    # ================= phase 1a: GAT =================
    with tc.tile_pool(name="p1a", bufs=1) as p1a, \
         tc.tile_pool(name="p1a_sm", bufs=3) as p1s, \
         tc.tile_pool(name="p1a_ps", bufs=4, space="PSUM") as pps, \
         tc.tile_pool(name="p1a_ps2", bufs=2, space="PSUM") as pps2:

        gatw_sb = p1a.tile([128, 2, H], f32)
        nc.sync.dma_start(out=gatw_sb[:], in_=d_gatw.rearrange("(k p) j -> p k j", p=128))
        a_bc = p1a.tile([128, 2 * H], f32)
        nc.gpsimd.dma_start(out=a_bc[:], in_=d_aflat.to_broadcast([128, 2 * H]))
        gamma_bc = p1a.tile([128, H], f32)
        nc.gpsimd.dma_start(out=gamma_bc[:], in_=d_gamma.to_broadcast([128, H]))
        beta_bc = None
        if has_beta:
            beta_bc = p1a.tile([128, H], f32)
            nc.gpsimd.dma_start(out=beta_bc[:], in_=d_beta.to_broadcast([128, H]))
        gbias_bc = None
        if has_gbias:
            gbias_bc = p1a.tile([128, H], f32)
            nc.gpsimd.dma_start(out=gbias_bc[:], in_=d_gbias.to_broadcast([128, H]))

        nodes = big.tile([128, BL, H], f32, tag="sB")     # route_emb as [l, b, h]
        nc.sync.dma_start(out=nodes[:], in_=d_remb.rearrange("b l h -> l b h"))

        xw_C = big.tile([128, BL, H], f32, tag="sA")      # xw as [l, b, h']

        # xw = (emb @ W): per sample transpose emb then matmul with W moving
        for b in range(BL):
            pt = pps.tile([128, 2, 128], f32, tag="ptr")
            for k in range(2):
                nc.tensor.transpose(
                    out=pt[:, k, :], in_=nodes[:, b, k * 128:(k + 1) * 128],
                    identity=ident[:],
                )
            embT = p1s.tile([128, 2, 128], f32, tag="embT")
            evac_copy(embT[:, 0, :], pt[:, 0, :])
            evac_copy(embT[:, 1, :], pt[:, 1, :])
            pxw = pps2.tile([128, H], f32, tag="pxw")
            for k in range(2):
                nc.tensor.matmul(
                    out=pxw[:], lhsT=embT[:, k, :], rhs=gatw_sb[:, k, :],
                    start=(k == 0), stop=(k == 1),
                )
            evac_copy(xw_C[:, b, :], pxw[:])

        # attention logits: a_s/a_d per node  [128 l, (b h)]
        a_s = p1a.tile([128, BL, HEADS], f32)
        a_d = p1a.tile([128, BL, HEADS], f32)
        for b in range(BL):
            xw_v = xw_C[:, b, :]
            prod = p1s.tile([128, 2, H], f32, tag="aprod")
            nc.vector.tensor_tensor(out=prod[:, 0, :], in0=xw_v, in1=a_bc[:, 0:H], op=Alu.mult)
            nc.vector.tensor_tensor(out=prod[:, 1, :], in0=xw_v, in1=a_bc[:, H:2 * H], op=Alu.mult)
            nc.vector.tensor_reduce(
                out=a_s[:, b, :], in_=prod[:, 0, :].rearrange("p (h d) -> p h d", h=HEADS),
                axis=AX.X, op=Alu.add,
            )
            nc.vector.tensor_reduce(
                out=a_d[:, b, :], in_=prod[:, 1, :].rearrange("p (h d) -> p h d", h=HEADS),
                axis=AX.X, op=Alu.add,
            )

        NF = BL * HEADS  # 128
        a_s_f = a_s[:].rearrange("p b h -> p (b h)")
        a_d_f = a_d[:].rearrange("p b h -> p (b h)")
        # shifted copies of a_s along l (partition axis) via sbuf->sbuf DMA
        # boundary rows hold -1e30 (memset covers a 32-partition block; the
        # shift-DMA then overwrites all but the boundary row)
        a_dn = p1a.tile([128, NF], f32)   # a_dn[l] = a_s[l-1]
        a_up = p1a.tile([128, NF], f32)   # a_up[l] = a_s[l+1]
        nc.vector.memset(a_dn[0:32, :], -1e30)
        nc.vector.memset(a_up[96:128, :], -1e30)
        nc.sync.dma_start(out=a_dn[1:128, :], in_=a_s_f[0:127, :])
        nc.sync.dma_start(out=a_up[0:127, :], in_=a_s_f[1:128, :])

        ef = p1a.tile([128, NF], f32)     # edge (l-1 -> l)
        eb = p1a.tile([128, NF], f32)     # edge (l+1 -> l)
        nc.vector.tensor_tensor(out=ef[:], in0=a_dn[:], in1=a_d_f, op=Alu.add)
        nc.vector.tensor_tensor(out=eb[:], in0=a_up[:], in1=a_d_f, op=Alu.add)
        nc.scalar.activation(out=ef[:], in_=ef[:], func=Act.Lrelu, alpha=0.2)
        nc.scalar.activation(out=eb[:], in_=eb[:], func=Act.Lrelu, alpha=0.2)

        mx = p1a.tile([128, NF], f32)
        nc.vector.tensor_tensor(out=mx[:], in0=ef[:], in1=eb[:], op=Alu.max)
        nc.vector.tensor_tensor(out=ef[:], in0=ef[:], in1=mx[:], op=Alu.subtract)
        nc.vector.tensor_tensor(out=eb[:], in0=eb[:], in1=mx[:], op=Alu.subtract)
        nc.scalar.activation(out=ef[:], in_=ef[:], func=Act.Exp)
        nc.scalar.activation(out=eb[:], in_=eb[:], func=Act.Exp)
        den = p1a.tile([128, NF], f32)
        nc.vector.tensor_tensor(out=den[:], in0=ef[:], in1=eb[:], op=Alu.add)
        nc.vector.reciprocal(out=den[:], in_=den[:])
        al_f = p1a.tile([128, NF], f32)
        al_b = p1a.tile([128, NF], f32)
        nc.vector.tensor_tensor(out=al_f[:], in0=ef[:], in1=den[:], op=Alu.mult)
        nc.vector.tensor_tensor(out=al_b[:], in0=eb[:], in1=den[:], op=Alu.mult)

        # aggregation + residual + LayerNorm per sample
        for b in range(BL):
            xw_dn = p1s.tile([128, H], f32, tag="xwdn")   # xw[l-1]
            xw_up = p1s.tile([128, H], f32, tag="xwup")   # xw[l+1]
            nc.vector.memset(xw_dn[0:32, :], 0.0)
            nc.vector.memset(xw_up[96:128, :], 0.0)
            nc.sync.dma_start(out=xw_dn[1:128, :], in_=xw_C[0:127, b, :])
            nc.sync.dma_start(out=xw_up[0:127, :], in_=xw_C[1:128, b, :])

            acc = p1s.tile([128, H], f32, tag="acc")
            for h in range(HEADS):
                hs = slice(h * DH, (h + 1) * DH)
                cf = al_f[:, b * HEADS + h: b * HEADS + h + 1]
                cb = al_b[:, b * HEADS + h: b * HEADS + h + 1]
                nc.vector.tensor_scalar_mul(out=acc[:, hs], in0=xw_dn[:, hs], scalar1=cf)
                nc.vector.scalar_tensor_tensor(
                    out=acc[:, hs], in0=xw_up[:, hs], scalar=cb,
                    in1=acc[:, hs], op0=Alu.mult, op1=Alu.add,
                )
            if has_gbias:
                nc.vector.tensor_tensor(out=acc[:], in0=acc[:], in1=gbias_bc[:], op=Alu.add)
            nc.vector.tensor_tensor(out=acc[:], in0=acc[:], in1=nodes[:, b, :], op=Alu.add)

            st6 = p1s.tile([128, 6], f32, tag="st6")
            nc.vector.bn_stats(out=st6[:], in_=acc[:])
            mv = p1s.tile([128, 2], f32, tag="mv")
            nc.vector.bn_aggr(out=mv[:], in_=st6[:])
            rstd = p1s.tile([128, 1], f32, tag="rstd")
            nc.scalar.activation(out=rstd[:], in_=mv[:, 1:2], func=Act.Sqrt, bias=eps_sb[:])
            nc.vector.reciprocal(out=rstd[:], in_=rstd[:])
            nc.vector.tensor_scalar(
                out=acc[:], in0=acc[:], scalar1=mv[:, 0:1], scalar2=rstd[:],
                op0=Alu.subtract, op1=Alu.mult,
            )
            nc.vector.tensor_tensor(out=ro_C[:, b, :], in0=acc[:], in1=gamma_bc[:], op=Alu.mult)
            if has_beta:
                nc.vector.tensor_tensor(out=ro_C[:, b, :], in0=ro_C[:, b, :], in1=beta_bc[:], op=Alu.add)

            # transpose route_outputs (scaled by 1/sqrt(H)) for the score matmuls
            ptr = pps.tile([128, 2, 128], f32, tag="ptr")
            for k in range(2):
                nc.tensor.transpose(
                    out=ptr[:, k, :], in_=ro_C[:, b, k * 128:(k + 1) * 128],
                    identity=ident[:],
                )
                evac_copy(roT[:, k, b, :], ptr[:, k, :], scale=1.0 / 16.0)

    # hidden0 = mean over l -> [128, 2, 32] (hT layout)
    h0_sb = consts.tile([128, 2, BL], f32)
    for k in range(2):
        red = consts.tile([128, BL], f32, tag=f"h0red{k}")
        nc.vector.tensor_reduce(out=red[:], in_=roT[:, k, :, :], axis=AX.X, op=Alu.add)
        nc.vector.tensor_scalar_mul(out=h0_sb[:, k, :], in0=red[:], scalar1=16.0 / 128.0)

    # ================= phase 1b: gather + gi =================
    et_view = d_et.rearrange("k p n -> p k n")   # [128, 2, NGT*128]
    with tc.tile_pool(name="p1b_big", bufs=1) as p1b, \
         tc.tile_pool(name="p1b_sm", bufs=3) as p1bs, \
         tc.tile_pool(name="p1b_st", bufs=3) as p1bst, \
         tc.tile_pool(name="p1b_rb", bufs=2) as p1brb, \
         tc.tile_pool(name="p1b_ps", bufs=4, space="PSUM") as bps, \
         tc.tile_pool(name="p1b_ps2", bufs=2, space="PSUM") as bps2:

        wih_sb = p1b.tile([128, 2, 3 * H], f32)
        nc.sync.dma_start(out=wih_sb[:], in_=d_wih.rearrange("(k p) j -> p k j", p=128))

        for j in range(NGT):
            e_t = p1bs.tile([128, H], f32, tag="gat")
            nc.gpsimd.indirect_dma_start(
                out=e_t[:], out_offset=None, in_=d_emb,
                in_offset=bass.IndirectOffsetOnAxis(ap=idx_sb[:, j:j + 1], axis=0),
            )
            pe = bps.tile([128, 2, 128], f32, tag="pet")
            etb = p1bs.tile([128, 2, 128], f32, tag="etb")
            for k in range(2):
                nc.tensor.transpose(
                    out=pe[:, k, :], in_=e_t[:, k * 128:(k + 1) * 128], identity=ident[:],
                )
                evac_copy(etb[:, k, :], pe[:, k, :])
            nc.sync.dma_start(out=et_view[:, :, j * 128:(j + 1) * 128], in_=etb[:])

        for c in range(NCH_GI):
            csl = slice(c * CS_GI, (c + 1) * CS_GI)
            rate_bc = p1brb.tile([128, CS_GI], f32, tag="ratebc")
            nc.gpsimd.dma_start(out=rate_bc[:], in_=d_rate[:, csl].to_broadcast([128, CS_GI]))
            ETc = p1brb.tile([128, 2, CS_GI], f32, tag="etc")
            nc.sync.dma_start(out=ETc[:], in_=et_view[:, :, csl])
            for m in range(G):
                msl = slice(m * 128, (m + 1) * 128)
                pgi = bps2.tile([128, CS_GI], f32, tag="pgi")
                for k in range(2):
                    nc.tensor.matmul(
                        out=pgi[:], lhsT=wih_sb[:, k, msl], rhs=ETc[:, k, :],
                        start=(k == 0), stop=(k == 1),
                    )
                stage = p1bst.tile([128, CS_GI], f32, tag="gstage")
                nc.vector.scalar_tensor_tensor(
                    out=stage[:], in0=rate_bc[:], scalar=ratecol_sb[:, m:m + 1],
                    in1=pgi[:], op0=Alu.mult, op1=Alu.add,
                )
                if (m < 4 and has_bias_rz) or (m >= 4 and has_bias_in):
                    bsl = brz_sb[:, m:m + 1] if m < 4 else bin_sb[:, m - 4:m - 3]
                    nc.vector.tensor_scalar_add(out=stage[:], in0=stage[:], scalar1=bsl)
                nc.sync.dma_start(out=gi_view[:, m, csl], in_=stage[:])

    # ================= phase 2: sequential GRU =================
    H_all = big.tile([128, 2, TOK], f32, tag="sB")
    H4 = H_all[:].rearrange("p k (s b) -> p k s b", b=BL)

    with tc.tile_pool(name="p2_gib", bufs=2) as pgib, \
         tc.tile_pool(name="p2_g", bufs=3) as pg, \
         tc.tile_pool(name="p2_rz", bufs=2, space="PSUM") as prz, \
         tc.tile_pool(name="p2_hn", bufs=2, space="PSUM") as phn:

        gib = None
        w_mm = whh_bf if rec_bf else whh_sb
        hprev_bf = None
        if rec_bf:
            hprev_bf = pg.tile([128, 2, BL], bf16, tag="hbf")
            nc.vector.tensor_copy(out=hprev_bf[:], in_=h0_sb[:])

        for s in range(NSTEP):
            sg = s % GIB
            if sg == 0:
                nb = min(GIB, NSTEP - s)
                gib = pgib.tile([128, G, GIB * BL], f32, tag="gib")
                nc.sync.dma_start(
                    out=gib[:, :, 0:nb * BL],
                    in_=gi_view[:, :, s * BL:(s + nb) * BL],
                )
            if rec_bf:
                rhs_t = hprev_bf
            elif s == 0:
                rhs_t = h0_sb
            else:
                rhs_t = None  # slice of H_all below

            p_rz = prz.tile([128, 128], f32, tag="prz")
            p_hn = phn.tile([128, 64], f32, tag="phn")
            for g in range(G):
                if g < 4:
                    out_ap = p_rz[:, g * BL:(g + 1) * BL]
                else:
                    out_ap = p_hn[:, (g - 4) * BL:(g - 3) * BL]
                gsl = slice(g * 128, (g + 1) * 128)
                for k in range(2):
                    if rhs_t is not None:
                        rhs = rhs_t[:, k, :]
                    else:
                        rhs = H_all[:, k, (s - 1) * BL:s * BL]
                    nc.tensor.matmul(
                        out=out_ap, lhsT=w_mm[:, k, gsl], rhs=rhs,
                        start=(k == 0), stop=(k == 1),
                    )

            gsl_rz = gib[:, 0:4, sg * BL:(sg + 1) * BL]
            gsl_n = gib[:, 4:6, sg * BL:(sg + 1) * BL]
            p_rz4 = p_rz[:].rearrange("p (g b) -> p g b", g=4)
            nc.vector.tensor_tensor(out=p_rz4, in0=p_rz4, in1=gsl_rz, op=Alu.add)
            rz = pg.tile([128, 128], f32, tag="rz")
            nc.scalar.activation(out=rz[:], in_=p_rz[:], func=Act.Sigmoid)

            # z-path (overlaps the n-path)
            omz = pg.tile([128, 64], f32, tag="omz")
            nc.vector.tensor_scalar(
                out=omz[:], in0=rz[:, 64:128], scalar1=-1.0, scalar2=1.0,
                op0=Alu.mult, op1=Alu.add,
            )
            zh = pg.tile([128, 2, BL], f32, tag="zh")
            if s == 0:
                hview = h0_sb[:]
            else:
                hview = H_all[:, :, (s - 1) * BL:s * BL]
            nc.vector.tensor_tensor(
                out=zh[:], in0=rz[:, 64:128].rearrange("p (k b) -> p k b", k=2),
                in1=hview, op=Alu.mult,
            )

            # n-path
            rhn = pg.tile([128, 2, BL], f32, tag="rhn")
            p_hn2 = p_hn[:].rearrange("p (k b) -> p k b", k=2)
            r2 = rz[:, 0:64].rearrange("p (k b) -> p k b", k=2)
            if has_bias_hn:
                for k in range(2):
                    nc.vector.scalar_tensor_tensor(
                        out=rhn[:, k, :], in0=p_hn2[:, k, :], scalar=bhn_sb[:, k:k + 1],
                        in1=r2[:, k, :], op0=Alu.add, op1=Alu.mult,
                    )
            else:
                nc.vector.tensor_tensor(out=rhn[:], in0=p_hn2, in1=r2, op=Alu.mult)
            nc.vector.tensor_tensor(out=rhn[:], in0=rhn[:], in1=gsl_n, op=Alu.add)
            n_t = pg.tile([128, 2, BL], f32, tag="nt")
            nc.scalar.activation(out=n_t[:], in_=rhn[:], func=Act.Tanh)

            # h' = (1-z)*n + z*h
            hnew = H_all[:, :, s * BL:(s + 1) * BL]
            nc.vector.tensor_tensor(
                out=hnew, in0=omz[:].rearrange("p (k b) -> p k b", k=2),
                in1=n_t[:], op=Alu.mult,
            )
            nc.vector.tensor_tensor(out=hnew, in0=hnew, in1=zh[:], op=Alu.add)
            if rec_bf:
                hprev_bf = pg.tile([128, 2, BL], bf16, tag="hbf")
                nc.vector.tensor_copy(out=hprev_bf[:], in_=hnew)

    # ================= phase 3a: attention =================
    wT = big.tile([128, 2, TOK], bf16, tag="sA")
    wT4 = wT[:].rearrange("p k (s b) -> p k s b", b=BL)

    with tc.tile_pool(name="p3a_sm", bufs=4) as p3s, \
         tc.tile_pool(name="p3a_ps", bufs=2, space="PSUM") as aps, \
         tc.tile_pool(name="p3a_pt", bufs=2, space="PSUM") as apt, \
         tc.tile_pool(name="p3a_pw", bufs=2, space="PSUM") as apw:

        for c in range(2):
            cs = 128 if c == 0 else NSTEP - 128
            for b in range(BL):
                ps = aps.tile([128, 128], f32, tag="ps")
                for k in range(2):
                    nc.tensor.matmul(
                        out=ps[0:cs, :],
                        lhsT=H4[:, k, c * 128:c * 128 + cs, b],
                        rhs=roT[:, k, b, :],
                        start=(k == 0), stop=(k == 1),
                    )
                mxs = p3s.tile([128, 1], f32, tag="mxs")
                nc.vector.tensor_reduce(out=mxs[0:cs, :], in_=ps[0:cs, :], axis=AX.X, op=Alu.max)
                nc.vector.tensor_scalar_mul(out=mxs[0:cs, :], in0=mxs[0:cs, :], scalar1=-1.0)
                attn = p3s.tile([128, 128], f32, tag="attn")
                dns = p3s.tile([128, 1], f32, tag="dns")
                nc.scalar.activation(
                    out=attn[0:cs, :], in_=ps[0:cs, :], func=Act.Exp,
                    bias=mxs[0:cs, :], accum_out=dns[0:cs, :],
                )
                nc.vector.reciprocal(out=dns[0:cs, :], in_=dns[0:cs, :])
                nc.vector.tensor_scalar_mul(out=attn[0:cs, :], in0=attn[0:cs, :], scalar1=dns[0:cs, :])
                nc.sync.dma_start(
                    out=d_oid[c * 128 + 1:c * 128 + 1 + cs, b, :], in_=attn[0:cs, :],
                )
                pt = apt.tile([128, 128], f32, tag="pt")
                nc.tensor.transpose(out=pt[:, 0:cs], in_=attn[0:cs, :], identity=ident[0:cs, 0:cs])
                aT = p3s.tile([128, 128], f32, tag="aT")
                evac_copy(aT[:, 0:cs], pt[:, 0:cs])
                for ko in range(2):
                    pw = apw.tile([128, 128], f32, tag="pw")
                    nc.tensor.matmul(
                        out=pw[:, 0:cs], lhsT=ro_C[:, b, ko * 128:(ko + 1) * 128],
                        rhs=aT[:, 0:cs], start=True, stop=True,
                    )
                    nc.vector.tensor_copy(
                        out=wT4[:, ko, c * 128:c * 128 + cs, b], in_=pw[:, 0:cs],
                    )

    # ================= phase 3b: rate head =================
    orate_flat = d_orate.rearrange("t b o -> o (t b)")   # [1, T*BL]
    with tc.tile_pool(name="p3b", bufs=1) as p3b, \
         tc.tile_pool(name="p3b_r1", bufs=6) as pr1, \
         tc.tile_pool(name="p3b_rt", bufs=2) as prt, \
         tc.tile_pool(name="p3b_ps", bufs=4, space="PSUM") as fps, \
         tc.tile_pool(name="p3b_f2", bufs=2, space="PSUM") as f2ps:

        fc1T_sb = p3b.tile([128, 4, 2 * H], f32)
        nc.sync.dma_start(out=fc1T_sb[:], in_=d_fc1.rearrange("(k p) j -> p k j", p=128))
        fc1T_bf = p3b.tile([128, 4, 2 * H], bf16)
        nc.vector.tensor_copy(out=fc1T_bf[:], in_=fc1T_sb[:])
        fc2T_bf = p3b.tile([128, 4], bf16)
        nc.vector.tensor_copy(out=fc2T_bf[:], in_=fc2T_sb[:])

        for cc in range(NCH_GI):
            csl = slice(cc * CS_GI, (cc + 1) * CS_GI)
            Hc_bf = prt.tile([128, 2, CS_GI], bf16, tag="hcbf")
            nc.vector.tensor_copy(out=Hc_bf[:], in_=H_all[:, :, csl])
            r1s = []
            for m in range(4):
                msl = slice(m * 128, (m + 1) * 128)
                pf1 = fps.tile([128, CS_GI], f32, tag="pf1")
                for k in range(4):
                    rhs = Hc_bf[:, k, :] if k < 2 else wT[:, k - 2, csl]
                    nc.tensor.matmul(
                        out=pf1[:], lhsT=fc1T_bf[:, k, msl], rhs=rhs,
                        start=(k == 0), stop=(k == 3),
                    )
                r1 = pr1.tile([128, CS_GI], bf16, tag="r1")
                nc.scalar.activation(
                    out=r1[:], in_=pf1[:], func=Act.Relu, bias=fc1b_sb[:, m:m + 1],
                )
                r1s.append(r1)
            pf2 = f2ps.tile([1, CS_GI], f32, tag="pf2")
            for m in range(4):
                nc.tensor.matmul(
                    out=pf2[:], lhsT=fc2T_bf[:, m:m + 1], rhs=r1s[m][:],
                    start=(m == 0), stop=(m == 3),
                )
            rate_sb = prt.tile([1, CS_GI], f32, tag="rchunk")
            nc.scalar.activation(
                out=rate_sb[:], in_=pf2[:], func=Act.Sigmoid, bias=fc2b_sb[:],
            )
            # out_rate flat index (t, b) = token index + BL (row 0 stays zero)
            nc.sync.dma_start(
                out=orate_flat[:, BL + cc * CS_GI: BL + (cc + 1) * CS_GI],
                in_=rate_sb[:],
            )


def build(flags_key):
    """Build + compile the Bass module (cached per flag set)."""
    if flags_key in _BUILD_CACHE:
        return _BUILD_CACHE[flags_key]
    import concourse.bass as bass
    import concourse.tile as tile
    from concourse import bacc, mybir

    flags = dict(flags_key)
    dt = mybir.dt
    nc = bacc.Bacc("TRN2", target_bir_lowering=False, debug=False, num_devices=NCORES)

    def din(name, shape, dtype=dt.float32):
        return nc.dram_tensor(name, list(shape), dtype, kind="ExternalInput")

    io = {}
    io["route_emb"] = din("route_emb", [BL, L, H])
    io["trg_idx"] = din("trg_idx", [128, NGT], dt.int32)
    io["trg_rate"] = din("trg_rate", [1, TOK])
    io["emb_id"] = din("emb_id", [ID, H])
    io["gat_w"] = din("gat_w", [H, H])
    io["gat_a"] = din("gat_a", [1, 2 * H])
    io["gat_bias"] = din("gat_bias", [1, H])
    io["ln_gamma"] = din("ln_gamma", [1, H])
    io["ln_beta"] = din("ln_beta", [1, H])
    io["w_hhT"] = din("w_hhT", [H, 3 * H])
    io["w_ihT"] = din("w_ihT", [H, 3 * H])
    io["rate_col"] = din("rate_col", [128, G])
    io["fc1_wT"] = din("fc1_wT", [2 * H, 2 * H])
    io["fc1_b"] = din("fc1_b", [128, 4])
    io["fc2_wT"] = din("fc2_wT", [128, 4])
    io["fc2_b"] = din("fc2_b", [1, 1])
    io["bias_rz"] = din("bias_rz", [128, 4])
    io["bias_in"] = din("bias_in", [128, 2])
    io["bias_hn"] = din("bias_hn", [128, 2])
    io["out_id"] = nc.dram_tensor("out_id", [T, BL, L], dt.float32, kind="ExternalOutput")
    io["out_rate"] = nc.dram_tensor("out_rate", [T, BL, 1], dt.float32, kind="ExternalOutput")
    io["gi_scratch"] = nc.dram_tensor("gi_scratch", [G, 128, TOK], dt.float32)
    io["et_scratch"] = nc.dram_tensor("et_scratch", [2, 128, NGT * 128], dt.float32)

    with tile.TileContext(nc) as tc, ExitStack() as ctx:
        _emit(ctx, tc, io, flags)
    nc.compile()
    _BUILD_CACHE[flags_key] = nc
    return nc


def _prep_host(inputs, flags):
    """Per-core input maps from full inputs."""
    f32 = np.float32
    remb = np.ascontiguousarray(inputs["route_emb"], dtype=f32)      # [B, L, H]
    trg_id = np.asarray(inputs["trg_id"])                            # [T, B] int
    trg_rate = np.ascontiguousarray(inputs["trg_rate"], dtype=f32)   # [T, B, 1]
    emb_id = np.ascontiguousarray(inputs["emb_id"], dtype=f32)
    w_ih = np.asarray(inputs["gru_w_ih"], dtype=f32)                 # [3H, H+1]
    w_hh = np.asarray(inputs["gru_w_hh"], dtype=f32)                 # [3H, H]
    b_ih = np.asarray(inputs["gru_b_ih"], dtype=f32)
    b_hh = np.asarray(inputs["gru_b_hh"], dtype=f32)
    a_src = np.asarray(inputs["gat_a_src"], dtype=f32)               # [HEADS, DH]
    a_dst = np.asarray(inputs["gat_a_dst"], dtype=f32)

    gat_a = np.concatenate([a_src.reshape(-1), a_dst.reshape(-1)])[None, :]  # [1, 2H]
    w_hhT = np.ascontiguousarray(w_hh.T)                             # [H, 3H]
    w_ihT = np.ascontiguousarray(w_ih[:, :H].T)                      # [H, 3H]
    rate_col = np.ascontiguousarray(w_ih[:, H].reshape(G, 128).T)    # [128, G]
    fc1_wT = np.ascontiguousarray(np.asarray(inputs["fc1_w"], dtype=f32).T)
    fc1_b = np.ascontiguousarray(np.asarray(inputs["fc1_b"], dtype=f32).reshape(4, 128).T)
    fc2_wT = np.ascontiguousarray(np.asarray(inputs["fc2_w"], dtype=f32).reshape(4, 128).T)
    fc2_b = np.asarray(inputs["fc2_b"], dtype=f32).reshape(1, 1)
    bias_rz_full = (b_ih + b_hh)[:512]
    bias_rz = np.ascontiguousarray(bias_rz_full.reshape(4, 128).T)
    bias_in = np.ascontiguousarray(b_ih[512:].reshape(2, 128).T)
    bias_hn = np.ascontiguousarray(b_hh[512:].reshape(2, 128).T)

    common = dict(
        emb_id=emb_id,
        gat_w=np.ascontiguousarray(np.asarray(inputs["gat_w"], dtype=f32)),
        gat_a=np.ascontiguousarray(gat_a, dtype=f32),
        gat_bias=np.asarray(inputs["gat_bias"], dtype=f32).reshape(1, H),
        ln_gamma=np.asarray(inputs["ln_gamma"], dtype=f32).reshape(1, H),
        ln_beta=np.asarray(inputs["ln_beta"], dtype=f32).reshape(1, H),
        w_hhT=w_hhT, w_ihT=w_ihT, rate_col=rate_col,
        fc1_wT=fc1_wT, fc1_b=fc1_b, fc2_wT=fc2_wT, fc2_b=fc2_b,
        bias_rz=np.ascontiguousarray(bias_rz, dtype=f32),
        bias_in=np.ascontiguousarray(bias_in, dtype=f32),
        bias_hn=np.ascontiguousarray(bias_hn, dtype=f32),
    )

    in_maps = []
    for c in range(NCORES):
        bs = slice(c * BL, (c + 1) * BL)
        tid = trg_id[:NSTEP, bs].astype(np.int64)          # [255, 32]
        idx_flat = tid.reshape(-1)                         # tok = s*BL + b
        idx_pad = np.zeros(NGT * 128, dtype=np.int32)
        idx_pad[:TOK] = idx_flat.astype(np.int32)
        trg_idx = np.ascontiguousarray(idx_pad.reshape(NGT, 128).T)  # [128, NGT]
        rate_flat = np.ascontiguousarray(
            trg_rate[:NSTEP, bs, 0].reshape(1, TOK), dtype=f32)
        m = dict(common)
        m.update(
            route_emb=np.ascontiguousarray(remb[bs]),
            trg_idx=trg_idx,
            trg_rate=rate_flat,
        )
        in_maps.append(m)

    flags.update(
        has_gat_bias=bool(np.any(inputs["gat_bias"])),
        has_ln_beta=bool(np.any(inputs["ln_beta"])),
        has_bias_rz=bool(np.any(bias_rz_full)),
        has_bias_in=bool(np.any(b_ih[512:])),
        has_bias_hn=bool(np.any(b_hh[512:])),
    )
    return in_maps


def run(inputs, trace=False, recur_bf16=False):
    from concourse.bass_utils import run_bass_kernel_spmd

    flags = dict(recur_bf16=recur_bf16)
    in_maps = _prep_host(inputs, flags)
    nc = build(tuple(sorted(flags.items())))
    res = run_bass_kernel_spmd(nc, in_maps, list(range(NCORES)), trace=trace)
    oid = np.concatenate([r["out_id"] for r in res.results], axis=1)
    orate = np.concatenate([r["out_rate"] for r in res.results], axis=1)
    return (oid.astype(np.float32), orate.astype(np.float32)), res


def kernel(**inputs):
    (oid, orate), _ = run(inputs, trace=False,
                          recur_bf16=bool(int(os.environ.get("K_RECUR_BF16", "0"))))
    return oid, orate
